# revision 21
# baseline (speedup 1.0000x reference)
"""Trainium2 Bass kernel for nn_ConcatHeadModule (pairwise MLP scores).

scores[i, j] = W_out . tanh(th[i] + tm[j] + hid2_bias) + out_bias
  th = tanh(xf @ W_foh + cat_bias[:H]) @ W_hid2[:H]
  tm = tanh(xf @ W_fom + cat_bias[H:]) @ W_hid2[H:]

Sharding: rows i split across 8 cores (128 rows each); everything else
replicated.

Device layout: hid2 (64) is stacked twice on SBUF partitions so one tanh
tile covers a pair of output rows (i, i+64). ACT fuses the per-pair th[i]
add via its per-partition bias operand and writes float32r (1 PE cycle/col).
The hid2 reduction runs on PE with a [128,16] stationary whose columns
one-hot route each pair's two output rows; 8 pairs accumulate into one
[16,1024] PSUM tile (zeros elsewhere), so the result sits dense on
partitions 0..15 and evacuates with a single cheap DVE op per group.
"""

import sys

sys.path.insert(0, "/opt/trn_rl_repo")

import numpy as np

import concourse.bass as bass
import concourse.tile as tile
from concourse import bacc, mybir
from concourse.bass_utils import run_bass_kernel_spmd

N = 1024          # nodes
F = 512           # 2 * LDIMS
H = 128           # hidden
D = 64            # hid2
NCORES = 8
R = N // NCORES   # rows per core = 128
NPAIR = R // 2    # row pairs per core = 64

F32 = mybir.dt.float32
F32R = mybir.dt.float32r
Tanh = mybir.ActivationFunctionType.Tanh

PAIRS_PER_GROUP = 8
NGROUPS = NPAIR // PAIRS_PER_GROUP


def _build_program(out_bias: float):
    nc = bacc.Bacc("TRN2", target_bir_lowering=False, debug=False,
                   num_devices=NCORES)

    xtm_d = nc.dram_tensor("xtm", [F, R], F32, kind="ExternalInput")
    wfoh_d = nc.dram_tensor("wfoh", [F, H], F32, kind="ExternalInput")
    wfom_d = nc.dram_tensor("wfom", [F, H], F32, kind="ExternalInput")
    cbh_d = nc.dram_tensor("cbh", [H, 1], F32, kind="ExternalInput")
    cbm_d = nc.dram_tensor("cbm", [H, 1], F32, kind="ExternalInput")
    h2bh_d = nc.dram_tensor("h2bh", [D, 1], F32, kind="ExternalInput")
    w2all_d = nc.dram_tensor("w2all", [2 * D, 16 * PAIRS_PER_GROUP], F32,
                             kind="ExternalInput")
    wh2t_d = nc.dram_tensor("wh2t", [H, D], F32, kind="ExternalInput")
    wh2b_d = nc.dram_tensor("wh2b", [H, D], F32, kind="ExternalInput")
    out_d = nc.dram_tensor("out", [R, N], F32, kind="ExternalOutput")

    with tile.TileContext(nc) as tc:
        with (
            tc.tile_pool(name="consts", bufs=1) as consts,
            tc.tile_pool(name="raws", bufs=3) as raws,
            tc.tile_pool(name="proj", bufs=1) as proj,
            tc.tile_pool(name="tanb", bufs=6) as tanp,
            tc.tile_pool(name="addb", bufs=2) as addp,
            tc.tile_pool(name="tanbB", bufs=2) as tanbp,
            tc.tile_pool(name="stage", bufs=2) as stagep,
            tc.tile_pool(name="ps", bufs=2, space="PSUM") as psum,
            tc.tile_pool(name="pscore", bufs=2, space="PSUM") as psump,
            tc.tile_pool(name="dram", bufs=1, space="DRAM") as dram,
        ):
            # ---- load inputs, round matmul operands to f32r (DVE copy) ----
            # Trigger the tanh ACT table load immediately (overlaps loads).
            warm = consts.tile([H, 1], F32, tag="warm")
            nc.vector.memset(warm[:], 0.0)
            nc.scalar.activation(warm[:], warm[:], Tanh)

            # Round-robin DMA loads over engine queues so transfers overlap.
            _engs = [nc.sync, nc.gpsimd]
            _eng_i = [0]

            def _dma(dst, src):
                e = _engs[_eng_i[0] % len(_engs)]
                _eng_i[0] += 1
                e.dma_start(dst, src)

            def load_rounded(name, dram, shape):
                raw = raws.tile(shape, F32, tag=f"raw_{name}")
                _dma(raw[:], dram)
                rnd = consts.tile(shape, F32R, tag=name)
                nc.vector.tensor_copy(rnd[:], raw[:])
                return rnd

            xtm = [load_rounded(f"xtm{q}", xtm_d[q * H:(q + 1) * H, :], [H, R])
                   for q in range(4)]
            wfom = [load_rounded(f"wfom{q}", wfom_d[q * H:(q + 1) * H, :],
                                 [H, H]) for q in range(4)]
            wfoh = [load_rounded(f"wfoh{q}", wfoh_d[q * H:(q + 1) * H, :],
                                 [H, H]) for q in range(4)]
            wh2t = load_rounded("wh2t", wh2t_d[:], [H, D])
            wh2b = load_rounded("wh2b", wh2b_d[:], [H, D])
            w2all = load_rounded("w2all", w2all_d[:],
                                 [2 * D, 16 * PAIRS_PER_GROUP])
            cbh = consts.tile([H, 1], F32, tag="cbh")
            _dma(cbh[:], cbh_d[:])
            cbm = consts.tile([H, 1], F32, tag="cbm")
            _dma(cbm[:], cbm_d[:])
            h2bh = consts.tile([D, 1], F32, tag="h2bh")
            _dma(h2bh[:], h2bh_d[:])

            # ---- projections (all PE work in f32r, outputs at base 0) ----
            # Each core projects only its OWN 128-node block; the [64,128]
            # tmT contributions are AllGathered (32KB) instead of every core
            # loading the full 2MB x and projecting all 1024 nodes.
            # modfovT for own nodes: tanh(W_fom^T @ xtm + cbm)  [H, R]
            tanhm_own = proj.tile([H, R], F32R, tag="tanhm_own")
            pmo = psum.tile([H, R], F32, tag="ps")
            for q in range(4):
                nc.tensor.matmul(pmo[:], wfom[q][:], xtm[q][:],
                                 start=(q == 0), stop=(q == 3))
            nc.scalar.activation(tanhm_own[:], pmo[:], Tanh, bias=cbm[:])
            # headfovT for own nodes: [H, R]
            tanhh = proj.tile([H, R], F32R, tag="tanhh")
            pm2 = psum.tile([H, R], F32, tag="ps")
            for q in range(4):
                nc.tensor.matmul(pm2[:], wfoh[q][:], xtm[q][:],
                                 start=(q == 0), stop=(q == 3))
            nc.scalar.activation(tanhh[:], pm2[:], Tanh, bias=cbh[:])

            # own tmT chunk [D, R] -> AllGather -> [NCORES*D, R]
            ptm = psum.tile([D, R], F32, tag="ps")
            nc.tensor.matmul(ptm[:], wh2b[:], tanhm_own[:],
                             start=True, stop=True)
            tm_own = proj.tile([D, R], F32, tag="tm_own")
            nc.vector.tensor_copy(tm_own[:], ptm[:])
            cg_in = dram.tile([D, R], F32, tag="cg_in")
            cg_out = dram.tile([NCORES * D, R], F32, tag="cg_out")
            nc.gpsimd.dma_start(cg_in[:], tm_own[:])
            nc.gpsimd.collective_compute(
                "AllGather", mybir.AluOpType.bypass,
                replica_groups=[list(range(NCORES))],
                ins=[cg_in[:].opt()], outs=[cg_out[:].opt()])
            # gather back: tm_g[d, 128c + n] = cg_out[64c + d, n]
            tm_g = proj.tile([D, N], F32, tag="tm_g")
            nc.sync.dma_start(
                tm_g[:].rearrange("d (c n) -> d c n", c=NCORES),
                cg_out[:].rearrange("(c d) n -> d c n", c=NCORES))
            tm_half = proj.tile([D, N], F32, tag="tm_half")
            nc.vector.tensor_scalar_add(tm_half[:], tm_g[:], h2bh[:])
            tm_tile = proj.tile([2 * D, N], F32, tag="tm_tile")
            nc.sync.dma_start(tm_tile[0:D, :], tm_half[:])
            nc.gpsimd.dma_start(tm_tile[D:2 * D, :], tm_half[:])

            # thT at base 0, then DMA the two row-halves into th_stack
            th_half = proj.tile([D, R], F32, tag="th_half")
            ps3 = psum.tile([D, R], F32, tag="ps")
            nc.tensor.matmul(ps3[:], wh2t[:], tanhh[:], start=True, stop=True)
            nc.vector.tensor_copy(th_half[:], ps3[:])
            th_stack = proj.tile([2 * D, NPAIR], F32, tag="th_stack")
            nc.sync.dma_start(th_stack[0:D, :], th_half[:, 0:NPAIR])
            nc.gpsimd.dma_start(th_stack[D:2 * D, :], th_half[:, NPAIR:R])

            # ---- main pair loop ----
            # group g covers pairs p = 8g+u -> rows {8g+u, 64+8g+u}.
            # PSUM row u = local row 8g+u (w=0), row 8+u = 64+8g+u (w=1).
            # First FUSED_GROUPS groups use ACT-fused bias adds (no DVE
            # dependency, so ACT starts immediately); later groups use DVE
            # pre-adds + 4-pair big-block tanh (903 vs 1042 ns/pair on ACT),
            # with DVE running ahead during the fused phase.
            FUSED_GROUPS = 2
            tm_tile2 = proj.tile([2 * D, N], F32, tag="tm_tile2")
            for g in range(NGROUPS):
                if g == 1:
                    # second tm copy so DVE pre-adds don't contend with ACT
                    # reads; emitted after group 0 so it doesn't delay the
                    # main-loop start
                    nc.sync.dma_start(tm_tile2[0:D, :], tm_half[:])
                    nc.gpsimd.dma_start(tm_tile2[D:2 * D, :], tm_half[:])
                pscore = psump.tile([16, N], F32, tag="pscore")
                if g == 0 or g == NGROUPS - 1:
                    for u in range(PAIRS_PER_GROUP):
                        p = g * PAIRS_PER_GROUP + u
                        tanb = tanp.tile([2 * D, N], F32R, tag="tanb")
                        nc.scalar.activation(tanb[:], tm_tile[:], Tanh,
                                             bias=th_stack[:, p:p + 1])
                        for jh in range(2):
                            mv = slice(jh * 512, (jh + 1) * 512)
                            nc.tensor.matmul(
                                pscore[:, mv], w2all[:, 16 * u:16 * (u + 1)],
                                tanb[:, mv],
                                start=(u == 0),
                                stop=(u == PAIRS_PER_GROUP - 1),
                                skip_group_check=True)
                else:
                    for blk in range(2):
                        addb = addp.tile([2 * D, 4 * N], F32, tag="addb")
                        tanbB = tanbp.tile([2 * D, 4 * N], F32R, tag="tanbB")
                        for k in range(4):
                            u = blk * 4 + k
                            p = g * PAIRS_PER_GROUP + u
                            nc.vector.tensor_scalar_add(
                                addb[:, k * N:(k + 1) * N], tm_tile2[:],
                                th_stack[:, p:p + 1])
                        nc.scalar.activation(tanbB[:], addb[:], Tanh)
                        for k in range(4):
                            u = blk * 4 + k
                            for jh in range(2):
                                mv = slice(k * N + jh * 512,
                                           k * N + (jh + 1) * 512)
                                nc.tensor.matmul(
                                    pscore[:, jh * 512:(jh + 1) * 512],
                                    w2all[:, 16 * u:16 * (u + 1)],
                                    tanbB[:, mv],
                                    start=(u == 0),
                                    stop=(u == PAIRS_PER_GROUP - 1),
                                    skip_group_check=True)
                stg = stagep.tile([16, N], F32, tag="stg")
                nc.vector.tensor_scalar_add(stg[:], pscore[:], out_bias)
                base = g * PAIRS_PER_GROUP
                nc.sync.dma_start(out_d[base:base + 8, :], stg[0:8, :])
                nc.sync.dma_start(out_d[64 + base:64 + base + 8, :],
                                  stg[8:16, :])

    nc.compile()
    return nc


def _make_in_maps(x, W_foh, W_fom, cat_bias, W_hid2, hid2_bias, W_out):
    xf = x.reshape(N, F)
    xt = np.ascontiguousarray(xf.T)                      # [F, N]
    cbh = np.ascontiguousarray(cat_bias[:H].reshape(H, 1))
    cbm = np.ascontiguousarray(cat_bias[H:].reshape(H, 1))
    h2bh = np.ascontiguousarray(hid2_bias.reshape(D, 1))
    # w2all[:, 16u + c]: c==u -> [W_out; 0] (row 8g+u), c==8+u -> [0; W_out]
    w2all = np.zeros((2 * D, 16 * PAIRS_PER_GROUP), dtype=np.float32)
    for u in range(PAIRS_PER_GROUP):
        w2all[:D, 16 * u + u] = W_out[:, 0]
        w2all[D:, 16 * u + 8 + u] = W_out[:, 0]
    wh2t = np.ascontiguousarray(W_hid2[:H])
    wh2b = np.ascontiguousarray(W_hid2[H:])
    in_maps = []
    for c in range(NCORES):
        in_maps.append({
            "xtm": np.ascontiguousarray(xt[:, c * R:(c + 1) * R]),
            "wfoh": W_foh,
            "wfom": W_fom,
            "cbh": cbh,
            "cbm": cbm,
            "h2bh": h2bh,
            "w2all": w2all,
            "wh2t": wh2t,
            "wh2b": wh2b,
        })
    return in_maps


def kernel(x, W_foh, W_fom, cat_bias, W_hid2, hid2_bias, W_out, out_bias):
    x = np.asarray(x, dtype=np.float32)
    W_foh = np.asarray(W_foh, dtype=np.float32)
    W_fom = np.asarray(W_fom, dtype=np.float32)
    cat_bias = np.asarray(cat_bias, dtype=np.float32)
    W_hid2 = np.asarray(W_hid2, dtype=np.float32)
    hid2_bias = np.asarray(hid2_bias, dtype=np.float32)
    W_out = np.asarray(W_out, dtype=np.float32)
    out_bias = np.asarray(out_bias, dtype=np.float32)

    nc = _build_program(float(out_bias[0]))
    in_maps = _make_in_maps(x, W_foh, W_fom, cat_bias, W_hid2, hid2_bias,
                            W_out)
    res = run_bass_kernel_spmd(nc, in_maps, list(range(NCORES)))
    out = np.concatenate([res.results[c]["out"] for c in range(NCORES)],
                         axis=0)
    return out.astype(np.float32)


if __name__ == "__main__":
    rng = np.random.default_rng(0)
    ins = {
        "x": rng.standard_normal((N, 2, F // 2), dtype=np.float32),
        "W_foh": rng.standard_normal((F, H), dtype=np.float32) * 0.05,
        "W_fom": rng.standard_normal((F, H), dtype=np.float32) * 0.05,
        "cat_bias": rng.standard_normal((2 * H,), dtype=np.float32) * 0.05,
        "W_hid2": rng.standard_normal((2 * H, D), dtype=np.float32) * 0.05,
        "hid2_bias": rng.standard_normal((D,), dtype=np.float32) * 0.05,
        "W_out": rng.standard_normal((D, 1), dtype=np.float32) * 0.05,
        "out_bias": rng.standard_normal((1,), dtype=np.float32) * 0.05,
    }
    out = kernel(**ins)
    print("out", out.shape, out.dtype, out[:2, :4])


# revision 22
# speedup vs baseline: 1.5031x; 1.5031x over previous
"""Trainium2 Bass kernel for nn_ConcatHeadModule (pairwise MLP scores).

scores[i, j] = W_out . tanh(th[i] + tm[j] + hid2_bias) + out_bias
  th = tanh(xf @ W_foh + cat_bias[:H]) @ W_hid2[:H]
  tm = tanh(xf @ W_fom + cat_bias[H:]) @ W_hid2[H:]

Sharding: rows i split across 8 cores (128 rows each); everything else
replicated.

Device layout: hid2 (64) is stacked twice on SBUF partitions so one tanh
tile covers a pair of output rows (i, i+64). ACT fuses the per-pair th[i]
add via its per-partition bias operand and writes float32r (1 PE cycle/col).
The hid2 reduction runs on PE with a [128,16] stationary whose columns
one-hot route each pair's two output rows; 8 pairs accumulate into one
[16,1024] PSUM tile (zeros elsewhere), so the result sits dense on
partitions 0..15 and evacuates with a single cheap DVE op per group.
"""

import sys

sys.path.insert(0, "/opt/trn_rl_repo")

import numpy as np

import concourse.bass as bass
import concourse.tile as tile
from concourse import bacc, mybir
from concourse.bass_utils import run_bass_kernel_spmd

N = 1024          # nodes
F = 512           # 2 * LDIMS
H = 128           # hidden
D = 64            # hid2
NCORES = 8
R = N // NCORES   # rows per core = 128
NPAIR = R // 2    # row pairs per core = 64

F32 = mybir.dt.float32
F32R = mybir.dt.float32r
Tanh = mybir.ActivationFunctionType.Tanh

PAIRS_PER_GROUP = 8
NGROUPS = NPAIR // PAIRS_PER_GROUP


def _build_program(out_bias: float):
    nc = bacc.Bacc("TRN2", target_bir_lowering=False, debug=False,
                   num_devices=NCORES)

    xt_d = nc.dram_tensor("xt", [F, N], F32, kind="ExternalInput")
    xtm_d = nc.dram_tensor("xtm", [F, R], F32, kind="ExternalInput")
    wfoh_d = nc.dram_tensor("wfoh", [F, H], F32, kind="ExternalInput")
    wfom_d = nc.dram_tensor("wfom", [F, H], F32, kind="ExternalInput")
    cbh_d = nc.dram_tensor("cbh", [H, 1], F32, kind="ExternalInput")
    cbm_d = nc.dram_tensor("cbm", [H, 1], F32, kind="ExternalInput")
    h2bh_d = nc.dram_tensor("h2bh", [D, 1], F32, kind="ExternalInput")
    w2all_d = nc.dram_tensor("w2all", [2 * D, 16 * PAIRS_PER_GROUP], F32,
                             kind="ExternalInput")
    wh2t_d = nc.dram_tensor("wh2t", [H, D], F32, kind="ExternalInput")
    wh2b_d = nc.dram_tensor("wh2b", [H, D], F32, kind="ExternalInput")
    out_d = nc.dram_tensor("out", [R, N], F32, kind="ExternalOutput")

    with tile.TileContext(nc) as tc:
        with (
            tc.tile_pool(name="consts", bufs=1) as consts,
            tc.tile_pool(name="raws", bufs=3) as raws,
            tc.tile_pool(name="proj", bufs=1) as proj,
            tc.tile_pool(name="tanb", bufs=6) as tanp,
            tc.tile_pool(name="addb", bufs=2) as addp,
            tc.tile_pool(name="tanbB", bufs=2) as tanbp,
            tc.tile_pool(name="stage", bufs=2) as stagep,
            tc.tile_pool(name="ps", bufs=2, space="PSUM") as psum,
            tc.tile_pool(name="pscore", bufs=2, space="PSUM") as psump,
        ):
            # ---- load inputs, round matmul operands to f32r (DVE copy) ----
            # Trigger the tanh ACT table load immediately (overlaps loads).
            warm = consts.tile([H, 1], F32, tag="warm")
            nc.vector.memset(warm[:], 0.0)
            nc.scalar.activation(warm[:], warm[:], Tanh)

            # Round-robin DMA loads over engine queues so transfers overlap.
            _engs = [nc.sync, nc.gpsimd]
            _eng_i = [0]

            def _dma(dst, src):
                e = _engs[_eng_i[0] % len(_engs)]
                _eng_i[0] += 1
                e.dma_start(dst, src)

            def load_rounded(name, dram, shape, eng=None):
                raw = raws.tile(shape, F32, tag=f"raw_{name}")
                if eng is None:
                    _dma(raw[:], dram)
                else:
                    eng.dma_start(raw[:], dram)
                rnd = consts.tile(shape, F32R, tag=name)
                nc.vector.tensor_copy(rnd[:], raw[:])
                return rnd

            xtb = [load_rounded(f"xtb{q}", xt_d[q * H:(q + 1) * H, :], [H, N])
                   for q in range(4)]
            xtm = [load_rounded(f"xtm{q}", xtm_d[q * H:(q + 1) * H, :],
                                 [H, R], eng=nc.scalar) for q in range(4)]
            wfom = [load_rounded(f"wfom{q}", wfom_d[q * H:(q + 1) * H, :],
                                 [H, H], eng=nc.scalar) for q in range(4)]
            wfoh = [load_rounded(f"wfoh{q}", wfoh_d[q * H:(q + 1) * H, :],
                                 [H, H], eng=nc.scalar) for q in range(4)]
            wh2t = load_rounded("wh2t", wh2t_d[:], [H, D], eng=nc.scalar)
            wh2b = load_rounded("wh2b", wh2b_d[:], [H, D], eng=nc.scalar)
            w2all = load_rounded("w2all", w2all_d[:],
                                 [2 * D, 16 * PAIRS_PER_GROUP], eng=nc.scalar)
            cbh = consts.tile([H, 1], F32, tag="cbh")
            nc.scalar.dma_start(cbh[:], cbh_d[:])
            cbm = consts.tile([H, 1], F32, tag="cbm")
            nc.scalar.dma_start(cbm[:], cbm_d[:])
            h2bh = consts.tile([D, 1], F32, tag="h2bh")
            nc.scalar.dma_start(h2bh[:], h2bh_d[:])

            # ---- projections (all PE work in f32r, outputs at base 0) ----
            # modfovT over all nodes: tanh(W_fom^T @ xf^T + cbm)  [H, N]
            tanhm = proj.tile([H, N], F32R, tag="tanhm")
            for jh in range(2):
                pm = psum.tile([H, 512], F32, tag="ps")
                mv = slice(jh * 512, (jh + 1) * 512)
                for q in range(4):
                    nc.tensor.matmul(pm[:], wfom[q][:], xtb[q][:, mv],
                                     start=(q == 0), stop=(q == 3))
                nc.scalar.activation(tanhm[:, mv], pm[:], Tanh, bias=cbm[:])
            # headfovT for this core's rows: [H, R]
            tanhh = proj.tile([H, R], F32R, tag="tanhh")
            pm2 = psum.tile([H, R], F32, tag="ps")
            for q in range(4):
                nc.tensor.matmul(pm2[:], wfoh[q][:], xtm[q][:],
                                 start=(q == 0), stop=(q == 3))
            nc.scalar.activation(tanhh[:], pm2[:], Tanh, bias=cbh[:])

            # tmT + hid2_bias once at base 0, then DMA into both halves
            tm_half = proj.tile([D, N], F32, tag="tm_half")
            pt = psum.tile([D, N], F32, tag="ps")
            for jh in range(2):
                mv = slice(jh * 512, (jh + 1) * 512)
                nc.tensor.matmul(pt[:, mv], wh2b[:], tanhm[:, mv],
                                 start=True, stop=True)
            nc.vector.tensor_scalar_add(tm_half[:], pt[:], h2bh[:])
            tm_tile = proj.tile([2 * D, N], F32, tag="tm_tile")
            nc.sync.dma_start(tm_tile[0:D, :], tm_half[:])
            nc.gpsimd.dma_start(tm_tile[D:2 * D, :], tm_half[:])

            # thT at base 0, then DMA the two row-halves into th_stack
            th_half = proj.tile([D, R], F32, tag="th_half")
            ps3 = psum.tile([D, R], F32, tag="ps")
            nc.tensor.matmul(ps3[:], wh2t[:], tanhh[:], start=True, stop=True)
            nc.vector.tensor_copy(th_half[:], ps3[:])
            th_stack = proj.tile([2 * D, NPAIR], F32, tag="th_stack")
            nc.sync.dma_start(th_stack[0:D, :], th_half[:, 0:NPAIR])
            nc.gpsimd.dma_start(th_stack[D:2 * D, :], th_half[:, NPAIR:R])

            # ---- main pair loop ----
            # group g covers pairs p = 8g+u -> rows {8g+u, 64+8g+u}.
            # PSUM row u = local row 8g+u (w=0), row 8+u = 64+8g+u (w=1).
            # First FUSED_GROUPS groups use ACT-fused bias adds (no DVE
            # dependency, so ACT starts immediately); later groups use DVE
            # pre-adds + 4-pair big-block tanh (903 vs 1042 ns/pair on ACT),
            # with DVE running ahead during the fused phase.
            FUSED_GROUPS = 2
            tm_tile2 = proj.tile([2 * D, N], F32, tag="tm_tile2")
            for g in range(NGROUPS):
                if g == 1:
                    # second tm copy so DVE pre-adds don't contend with ACT
                    # reads; emitted after group 0 so it doesn't delay the
                    # main-loop start
                    nc.sync.dma_start(tm_tile2[0:D, :], tm_half[:])
                    nc.gpsimd.dma_start(tm_tile2[D:2 * D, :], tm_half[:])
                pscore = psump.tile([16, N], F32, tag="pscore")
                if g == 0 or g == NGROUPS - 1:
                    for u in range(PAIRS_PER_GROUP):
                        p = g * PAIRS_PER_GROUP + u
                        tanb = tanp.tile([2 * D, N], F32R, tag="tanb")
                        nc.scalar.activation(tanb[:], tm_tile[:], Tanh,
                                             bias=th_stack[:, p:p + 1])
                        for jh in range(2):
                            mv = slice(jh * 512, (jh + 1) * 512)
                            nc.tensor.matmul(
                                pscore[:, mv], w2all[:, 16 * u:16 * (u + 1)],
                                tanb[:, mv],
                                start=(u == 0),
                                stop=(u == PAIRS_PER_GROUP - 1),
                                skip_group_check=True)
                else:
                    for blk in range(2):
                        addb = addp.tile([2 * D, 4 * N], F32, tag="addb")
                        tanbB = tanbp.tile([2 * D, 4 * N], F32R, tag="tanbB")
                        for k in range(4):
                            u = blk * 4 + k
                            p = g * PAIRS_PER_GROUP + u
                            nc.vector.tensor_scalar_add(
                                addb[:, k * N:(k + 1) * N], tm_tile2[:],
                                th_stack[:, p:p + 1])
                        nc.scalar.activation(tanbB[:], addb[:], Tanh)
                        for k in range(4):
                            u = blk * 4 + k
                            for jh in range(2):
                                mv = slice(k * N + jh * 512,
                                           k * N + (jh + 1) * 512)
                                nc.tensor.matmul(
                                    pscore[:, jh * 512:(jh + 1) * 512],
                                    w2all[:, 16 * u:16 * (u + 1)],
                                    tanbB[:, mv],
                                    start=(u == 0),
                                    stop=(u == PAIRS_PER_GROUP - 1),
                                    skip_group_check=True)
                stg = stagep.tile([16, N], F32, tag="stg")
                nc.vector.tensor_scalar_add(stg[:], pscore[:], out_bias)
                base = g * PAIRS_PER_GROUP
                nc.sync.dma_start(out_d[base:base + 8, :], stg[0:8, :])
                nc.sync.dma_start(out_d[64 + base:64 + base + 8, :],
                                  stg[8:16, :])

    nc.compile()
    return nc


def _make_in_maps(x, W_foh, W_fom, cat_bias, W_hid2, hid2_bias, W_out):
    xf = x.reshape(N, F)
    xt = np.ascontiguousarray(xf.T)                      # [F, N]
    cbh = np.ascontiguousarray(cat_bias[:H].reshape(H, 1))
    cbm = np.ascontiguousarray(cat_bias[H:].reshape(H, 1))
    h2bh = np.ascontiguousarray(hid2_bias.reshape(D, 1))
    # w2all[:, 16u + c]: c==u -> [W_out; 0] (row 8g+u), c==8+u -> [0; W_out]
    w2all = np.zeros((2 * D, 16 * PAIRS_PER_GROUP), dtype=np.float32)
    for u in range(PAIRS_PER_GROUP):
        w2all[:D, 16 * u + u] = W_out[:, 0]
        w2all[D:, 16 * u + 8 + u] = W_out[:, 0]
    wh2t = np.ascontiguousarray(W_hid2[:H])
    wh2b = np.ascontiguousarray(W_hid2[H:])
    in_maps = []
    for c in range(NCORES):
        in_maps.append({
            "xt": xt,
            "xtm": np.ascontiguousarray(xt[:, c * R:(c + 1) * R]),
            "wfoh": W_foh,
            "wfom": W_fom,
            "cbh": cbh,
            "cbm": cbm,
            "h2bh": h2bh,
            "w2all": w2all,
            "wh2t": wh2t,
            "wh2b": wh2b,
        })
    return in_maps


def kernel(x, W_foh, W_fom, cat_bias, W_hid2, hid2_bias, W_out, out_bias):
    x = np.asarray(x, dtype=np.float32)
    W_foh = np.asarray(W_foh, dtype=np.float32)
    W_fom = np.asarray(W_fom, dtype=np.float32)
    cat_bias = np.asarray(cat_bias, dtype=np.float32)
    W_hid2 = np.asarray(W_hid2, dtype=np.float32)
    hid2_bias = np.asarray(hid2_bias, dtype=np.float32)
    W_out = np.asarray(W_out, dtype=np.float32)
    out_bias = np.asarray(out_bias, dtype=np.float32)

    nc = _build_program(float(out_bias[0]))
    in_maps = _make_in_maps(x, W_foh, W_fom, cat_bias, W_hid2, hid2_bias,
                            W_out)
    res = run_bass_kernel_spmd(nc, in_maps, list(range(NCORES)))
    out = np.concatenate([res.results[c]["out"] for c in range(NCORES)],
                         axis=0)
    return out.astype(np.float32)


if __name__ == "__main__":
    rng = np.random.default_rng(0)
    ins = {
        "x": rng.standard_normal((N, 2, F // 2), dtype=np.float32),
        "W_foh": rng.standard_normal((F, H), dtype=np.float32) * 0.05,
        "W_fom": rng.standard_normal((F, H), dtype=np.float32) * 0.05,
        "cat_bias": rng.standard_normal((2 * H,), dtype=np.float32) * 0.05,
        "W_hid2": rng.standard_normal((2 * H, D), dtype=np.float32) * 0.05,
        "hid2_bias": rng.standard_normal((D,), dtype=np.float32) * 0.05,
        "W_out": rng.standard_normal((D, 1), dtype=np.float32) * 0.05,
        "out_bias": rng.standard_normal((1,), dtype=np.float32) * 0.05,
    }
    out = kernel(**ins)
    print("out", out.shape, out.dtype, out[:2, :4])


# revision 23
# speedup vs baseline: 1.5351x; 1.0213x over previous
"""Trainium2 Bass kernel for nn_ConcatHeadModule (pairwise MLP scores).

scores[i, j] = W_out . tanh(th[i] + tm[j] + hid2_bias) + out_bias
  th = tanh(xf @ W_foh + cat_bias[:H]) @ W_hid2[:H]
  tm = tanh(xf @ W_fom + cat_bias[H:]) @ W_hid2[H:]

Sharding: rows i split across 8 cores (128 rows each); everything else
replicated.

Device layout: hid2 (64) is stacked twice on SBUF partitions so one tanh
tile covers a pair of output rows (i, i+64). ACT fuses the per-pair th[i]
add via its per-partition bias operand and writes float32r (1 PE cycle/col).
The hid2 reduction runs on PE with a [128,16] stationary whose columns
one-hot route each pair's two output rows; 8 pairs accumulate into one
[16,1024] PSUM tile (zeros elsewhere), so the result sits dense on
partitions 0..15 and evacuates with a single cheap DVE op per group.
"""

import sys

sys.path.insert(0, "/opt/trn_rl_repo")

import numpy as np

import concourse.bass as bass
import concourse.tile as tile
from concourse import bacc, mybir
from concourse.bass_utils import run_bass_kernel_spmd

N = 1024          # nodes
F = 512           # 2 * LDIMS
H = 128           # hidden
D = 64            # hid2
NCORES = 8
R = N // NCORES   # rows per core = 128
NPAIR = R // 2    # row pairs per core = 64

F32 = mybir.dt.float32
F32R = mybir.dt.float32r
Tanh = mybir.ActivationFunctionType.Tanh

PAIRS_PER_GROUP = 8
NGROUPS = NPAIR // PAIRS_PER_GROUP


def _build_program(out_bias: float):
    nc = bacc.Bacc("TRN2", target_bir_lowering=False, debug=False,
                   num_devices=NCORES)

    xt_d = nc.dram_tensor("xt", [F, N], F32, kind="ExternalInput")
    xtm_d = nc.dram_tensor("xtm", [F, R], F32, kind="ExternalInput")
    wfoh_d = nc.dram_tensor("wfoh", [F, H], F32, kind="ExternalInput")
    wfom_d = nc.dram_tensor("wfom", [F, H], F32, kind="ExternalInput")
    cbh_d = nc.dram_tensor("cbh", [H, 1], F32, kind="ExternalInput")
    cbm_d = nc.dram_tensor("cbm", [H, 1], F32, kind="ExternalInput")
    h2bh_d = nc.dram_tensor("h2bh", [D, 1], F32, kind="ExternalInput")
    w2all_d = nc.dram_tensor("w2all", [2 * D, 16 * PAIRS_PER_GROUP], F32,
                             kind="ExternalInput")
    wh2t_d = nc.dram_tensor("wh2t", [H, D], F32, kind="ExternalInput")
    wh2b_d = nc.dram_tensor("wh2b", [H, D], F32, kind="ExternalInput")
    out_d = nc.dram_tensor("out", [R, N], F32, kind="ExternalOutput")

    with tile.TileContext(nc) as tc:
        with (
            tc.tile_pool(name="consts", bufs=1) as consts,
            tc.tile_pool(name="raws", bufs=3) as raws,
            tc.tile_pool(name="proj", bufs=1) as proj,
            tc.tile_pool(name="tanb", bufs=6) as tanp,
            tc.tile_pool(name="addb", bufs=2) as addp,
            tc.tile_pool(name="tanbB", bufs=2) as tanbp,
            tc.tile_pool(name="stage", bufs=2) as stagep,
            tc.tile_pool(name="ps", bufs=2, space="PSUM") as psum,
            tc.tile_pool(name="pscore", bufs=2, space="PSUM") as psump,
        ):
            # ---- load inputs, round matmul operands to f32r (DVE copy) ----
            # Trigger the tanh ACT table load immediately (overlaps loads).
            warm = consts.tile([H, 1], F32, tag="warm")
            nc.vector.memset(warm[:], 0.0)
            nc.scalar.activation(warm[:], warm[:], Tanh)

            # Round-robin DMA loads over engine queues so transfers overlap.
            _engs = [nc.sync, nc.gpsimd]
            _eng_i = [0]

            def _dma(dst, src):
                e = _engs[_eng_i[0] % len(_engs)]
                _eng_i[0] += 1
                e.dma_start(dst, src)

            def load_rounded(name, dram, shape):
                raw = raws.tile(shape, F32, tag=f"raw_{name}")
                _dma(raw[:], dram)
                rnd = consts.tile(shape, F32R, tag=name)
                nc.vector.tensor_copy(rnd[:], raw[:])
                return rnd

            xtb = [load_rounded(f"xtb{q}", xt_d[q * H:(q + 1) * H, :], [H, N])
                   for q in range(4)]
            xtm = [load_rounded(f"xtm{q}", xtm_d[q * H:(q + 1) * H, :], [H, R])
                   for q in range(4)]
            wfom = [load_rounded(f"wfom{q}", wfom_d[q * H:(q + 1) * H, :],
                                 [H, H]) for q in range(4)]
            wfoh = [load_rounded(f"wfoh{q}", wfoh_d[q * H:(q + 1) * H, :],
                                 [H, H]) for q in range(4)]
            wh2t = load_rounded("wh2t", wh2t_d[:], [H, D])
            wh2b = load_rounded("wh2b", wh2b_d[:], [H, D])
            w2all = load_rounded("w2all", w2all_d[:],
                                 [2 * D, 16 * PAIRS_PER_GROUP])
            cbh = consts.tile([H, 1], F32, tag="cbh")
            _dma(cbh[:], cbh_d[:])
            cbm = consts.tile([H, 1], F32, tag="cbm")
            _dma(cbm[:], cbm_d[:])
            h2bh = consts.tile([D, 1], F32, tag="h2bh")
            _dma(h2bh[:], h2bh_d[:])

            # ---- projections (all PE work in f32r, outputs at base 0) ----
            # modfovT over all nodes: tanh(W_fom^T @ xf^T + cbm)  [H, N]
            tanhm = proj.tile([H, N], F32R, tag="tanhm")
            for jh in range(2):
                pm = psum.tile([H, 512], F32, tag="ps")
                mv = slice(jh * 512, (jh + 1) * 512)
                for q in range(4):
                    nc.tensor.matmul(pm[:], wfom[q][:], xtb[q][:, mv],
                                     start=(q == 0), stop=(q == 3))
                nc.scalar.activation(tanhm[:, mv], pm[:], Tanh, bias=cbm[:])
            # headfovT for this core's rows: [H, R]
            tanhh = proj.tile([H, R], F32R, tag="tanhh")
            pm2 = psum.tile([H, R], F32, tag="ps")
            for q in range(4):
                nc.tensor.matmul(pm2[:], wfoh[q][:], xtm[q][:],
                                 start=(q == 0), stop=(q == 3))
            nc.scalar.activation(tanhh[:], pm2[:], Tanh, bias=cbh[:])

            # tmT + hid2_bias once at base 0, then DMA into both halves
            tm_half = proj.tile([D, N], F32, tag="tm_half")
            pt = psum.tile([D, N], F32, tag="ps")
            for jh in range(2):
                mv = slice(jh * 512, (jh + 1) * 512)
                nc.tensor.matmul(pt[:, mv], wh2b[:], tanhm[:, mv],
                                 start=True, stop=True)
            nc.vector.tensor_scalar_add(tm_half[:], pt[:], h2bh[:])
            tm_tile = proj.tile([2 * D, N], F32, tag="tm_tile")
            nc.sync.dma_start(tm_tile[0:D, :], tm_half[:])
            nc.gpsimd.dma_start(tm_tile[D:2 * D, :], tm_half[:])

            # thT at base 0, then DMA the two row-halves into th_stack
            th_half = proj.tile([D, R], F32, tag="th_half")
            ps3 = psum.tile([D, R], F32, tag="ps")
            nc.tensor.matmul(ps3[:], wh2t[:], tanhh[:], start=True, stop=True)
            nc.vector.tensor_copy(th_half[:], ps3[:])
            th_stack = proj.tile([2 * D, NPAIR], F32, tag="th_stack")
            nc.sync.dma_start(th_stack[0:D, :], th_half[:, 0:NPAIR])
            nc.gpsimd.dma_start(th_stack[D:2 * D, :], th_half[:, NPAIR:R])

            # ---- main pair loop ----
            # group g covers pairs p = 8g+u -> rows {8g+u, 64+8g+u}.
            # PSUM row u = local row 8g+u (w=0), row 8+u = 64+8g+u (w=1).
            # First FUSED_GROUPS groups use ACT-fused bias adds (no DVE
            # dependency, so ACT starts immediately); later groups use DVE
            # pre-adds + 4-pair big-block tanh (903 vs 1042 ns/pair on ACT),
            # with DVE running ahead during the fused phase.
            FUSED_GROUPS = 2
            tm_tile2 = proj.tile([2 * D, N], F32, tag="tm_tile2")
            for g in range(NGROUPS):
                if g == 1:
                    # second tm copy so DVE pre-adds don't contend with ACT
                    # reads; emitted after group 0 so it doesn't delay the
                    # main-loop start
                    nc.sync.dma_start(tm_tile2[0:D, :], tm_half[:])
                    nc.gpsimd.dma_start(tm_tile2[D:2 * D, :], tm_half[:])
                pscore = psump.tile([16, N], F32, tag="pscore")
                if g == 0 or g == NGROUPS - 1:
                    for u in range(PAIRS_PER_GROUP):
                        p = g * PAIRS_PER_GROUP + u
                        tanb = tanp.tile([2 * D, N], F32R, tag="tanb")
                        nc.scalar.activation(tanb[:], tm_tile[:], Tanh,
                                             bias=th_stack[:, p:p + 1])
                        for jh in range(2):
                            mv = slice(jh * 512, (jh + 1) * 512)
                            nc.tensor.matmul(
                                pscore[:, mv], w2all[:, 16 * u:16 * (u + 1)],
                                tanb[:, mv],
                                start=(u == 0),
                                stop=(u == PAIRS_PER_GROUP - 1),
                                skip_group_check=True)
                else:
                    for blk in range(2):
                        addb = addp.tile([2 * D, 4 * N], F32, tag="addb")
                        tanbB = tanbp.tile([2 * D, 4 * N], F32R, tag="tanbB")
                        for k in range(4):
                            u = blk * 4 + k
                            p = g * PAIRS_PER_GROUP + u
                            nc.vector.tensor_scalar_add(
                                addb[:, k * N:(k + 1) * N], tm_tile2[:],
                                th_stack[:, p:p + 1])
                        nc.scalar.activation(tanbB[:], addb[:], Tanh)
                        for k in range(4):
                            u = blk * 4 + k
                            for jh in range(2):
                                mv = slice(k * N + jh * 512,
                                           k * N + (jh + 1) * 512)
                                nc.tensor.matmul(
                                    pscore[:, jh * 512:(jh + 1) * 512],
                                    w2all[:, 16 * u:16 * (u + 1)],
                                    tanbB[:, mv],
                                    start=(u == 0),
                                    stop=(u == PAIRS_PER_GROUP - 1),
                                    skip_group_check=True)
                stg = stagep.tile([16, N], F32, tag="stg")
                nc.vector.tensor_scalar_add(stg[:], pscore[:], out_bias)
                base = g * PAIRS_PER_GROUP
                nc.sync.dma_start(out_d[base:base + 8, :], stg[0:8, :])
                nc.sync.dma_start(out_d[64 + base:64 + base + 8, :],
                                  stg[8:16, :])

    nc.compile()
    return nc


def _make_in_maps(x, W_foh, W_fom, cat_bias, W_hid2, hid2_bias, W_out):
    xf = x.reshape(N, F)
    xt = np.ascontiguousarray(xf.T)                      # [F, N]
    cbh = np.ascontiguousarray(cat_bias[:H].reshape(H, 1))
    cbm = np.ascontiguousarray(cat_bias[H:].reshape(H, 1))
    h2bh = np.ascontiguousarray(hid2_bias.reshape(D, 1))
    # w2all[:, 16u + c]: c==u -> [W_out; 0] (row 8g+u), c==8+u -> [0; W_out]
    w2all = np.zeros((2 * D, 16 * PAIRS_PER_GROUP), dtype=np.float32)
    for u in range(PAIRS_PER_GROUP):
        w2all[:D, 16 * u + u] = W_out[:, 0]
        w2all[D:, 16 * u + 8 + u] = W_out[:, 0]
    wh2t = np.ascontiguousarray(W_hid2[:H])
    wh2b = np.ascontiguousarray(W_hid2[H:])
    in_maps = []
    for c in range(NCORES):
        in_maps.append({
            "xt": xt,
            "xtm": np.ascontiguousarray(xt[:, c * R:(c + 1) * R]),
            "wfoh": W_foh,
            "wfom": W_fom,
            "cbh": cbh,
            "cbm": cbm,
            "h2bh": h2bh,
            "w2all": w2all,
            "wh2t": wh2t,
            "wh2b": wh2b,
        })
    return in_maps


def kernel(x, W_foh, W_fom, cat_bias, W_hid2, hid2_bias, W_out, out_bias):
    x = np.asarray(x, dtype=np.float32)
    W_foh = np.asarray(W_foh, dtype=np.float32)
    W_fom = np.asarray(W_fom, dtype=np.float32)
    cat_bias = np.asarray(cat_bias, dtype=np.float32)
    W_hid2 = np.asarray(W_hid2, dtype=np.float32)
    hid2_bias = np.asarray(hid2_bias, dtype=np.float32)
    W_out = np.asarray(W_out, dtype=np.float32)
    out_bias = np.asarray(out_bias, dtype=np.float32)

    nc = _build_program(float(out_bias[0]))
    in_maps = _make_in_maps(x, W_foh, W_fom, cat_bias, W_hid2, hid2_bias,
                            W_out)
    res = run_bass_kernel_spmd(nc, in_maps, list(range(NCORES)))
    out = np.concatenate([res.results[c]["out"] for c in range(NCORES)],
                         axis=0)
    return out.astype(np.float32)


if __name__ == "__main__":
    rng = np.random.default_rng(0)
    ins = {
        "x": rng.standard_normal((N, 2, F // 2), dtype=np.float32),
        "W_foh": rng.standard_normal((F, H), dtype=np.float32) * 0.05,
        "W_fom": rng.standard_normal((F, H), dtype=np.float32) * 0.05,
        "cat_bias": rng.standard_normal((2 * H,), dtype=np.float32) * 0.05,
        "W_hid2": rng.standard_normal((2 * H, D), dtype=np.float32) * 0.05,
        "hid2_bias": rng.standard_normal((D,), dtype=np.float32) * 0.05,
        "W_out": rng.standard_normal((D, 1), dtype=np.float32) * 0.05,
        "out_bias": rng.standard_normal((1,), dtype=np.float32) * 0.05,
    }
    out = kernel(**ins)
    print("out", out.shape, out.dtype, out[:2, :4])
